# revision 1
# baseline (speedup 1.0000x reference)
"""DAS (delay-and-sum) beamforming kernel for 8 Trainium2 NeuronCores.

Strategy
--------
image[b,c,p] = sum_s sensor_data[b,c,s, t[s,p]]  with t a per-(sensor,pixel)
delay index into the 2048-sample trace, t in [0, 1867).

Sharding: sensors are split 16 per NeuronCore (8 cores x 16 = 128); each core
computes a partial image [8ch, 512*512] (8 channels = 4 batch x 2 comp) summed
over its 16 sensors; the host sums the 8 partial images (the unshard step).

Per core, for each 1024-pixel tile:
 - GPSIMD IndirectCopy gathers, for 8 sensors in parallel (one per Q7 core),
   1024 pixels from each sensor's 8 channel traces (the 16 SBUF partitions of
   a Q7 core share one index stream; 8 carry that sensor's channel traces).
   Two calls cover the core's 16 sensors (two groups of 8).
 - A [128->8] fp32 matmul on the PE (weights select partition j==c of each
   16-row group) reduces the 128 gathered rows to the 8 channel sums,
   accumulating both sensor groups in PSUM rows 0-7.
 - ACT copies PSUM [8,512]x2 into an [8, 16384] staging tile (partition
   starts equal and 0 => legal), and every 16 tiles the staging block is
   DMA'd to the HBM output slice (double-buffered).

Delay indices are computed on the host in numpy float32 with the exact op
sequence of the reference's _delay_indices; this is bit-identical to the
reference evaluated with jax on CPU (verified), honoring the truncating
int cast the reference documents.
"""

import numpy as np

import concourse.bass as bass
import concourse.mybir as mybir
from concourse.bass_utils import run_bass_kernel_spmd

F32 = mybir.dt.float32
U16 = mybir.dt.uint16

NX, NY = 512, 512
DX, DY = 1e-4, 1e-4
VS = 1550.0
DT = 2.5e-8

S = 128            # sensors
T = 2048           # trace length
NPX = NX * NY      # 262144 pixels
NCORES = 8
SPC = 16           # sensors per NeuronCore
TILE = 1024        # pixels per indirect_copy call (ISA cap: 1024 dst elems)
NTILES = NPX // TILE          # 256
BLK = 8                       # tiles per staging block
NBLK = NTILES // BLK          # 32 staging blocks
STG_F = BLK * TILE            # 8192 staging elems per partition
IDX_F = NPX // 16             # 16384 idx elems per partition per group


def _delay_indices(sensor_xy: np.ndarray) -> np.ndarray:
    """Replicates the reference's jax ops in numpy float32 (bit-identical on
    CPU: sub, add, mul, square, sqrt, div are all correctly rounded)."""
    ix = np.arange(1, NX + 1, dtype=np.float32)
    iy = np.arange(1, NY + 1, dtype=np.float32)
    x = sensor_xy[:, 0].astype(np.float32)
    y = sensor_xy[:, 1].astype(np.float32)
    dx = (x[:, None] - ix[None, :] + np.float32(1.0)) * np.float32(DX)
    dy = (y[:, None] - iy[None, :] + np.float32(1.0)) * np.float32(DY)
    dis = np.sqrt(dx[:, :, None] ** 2 + dy[:, None, :] ** 2)
    t = (dis / np.float32(VS) / np.float32(DT)).astype(np.int32)
    return t.reshape(sensor_xy.shape[0], -1)


def build_nc(repeat: int = 1, nv: int = TILE) -> bass.Bass:
    nc = bass.Bass()
    w = nc.declare_dram_parameter("w", [128, 8], F32, isOutput=False)
    d0 = nc.declare_dram_parameter("d0", [128, T], F32, isOutput=False)
    d1 = nc.declare_dram_parameter("d1", [128, T], F32, isOutput=False)
    idx = nc.declare_dram_parameter("idx", [128, 2 * IDX_F], U16, isOutput=False)
    out = nc.declare_dram_parameter("out", [8, NPX], F32, isOutput=True)

    with (
        nc.sbuf_tensor("w_sb", [128, 8], F32) as w_sb,
        nc.sbuf_tensor("d0_sb", [128, T], F32) as d0_sb,
        nc.sbuf_tensor("d1_sb", [128, T], F32) as d1_sb,
        nc.sbuf_tensor("i_sb", [128, 2 * IDX_F], U16) as i_sb,
        nc.sbuf_tensor("oA0", [128, TILE], F32) as oA0,
        nc.sbuf_tensor("oA1", [128, TILE], F32) as oA1,
        nc.sbuf_tensor("oB0", [128, TILE], F32) as oB0,
        nc.sbuf_tensor("oB1", [128, TILE], F32) as oB1,
        nc.sbuf_tensor("stg0", [8, STG_F], F32) as stg0,
        nc.sbuf_tensor("stg1", [8, STG_F], F32) as stg1,
        nc.psum_tensor("ps00", [8, 512], F32) as ps00,
        nc.psum_tensor("ps01", [8, 512], F32) as ps01,
        nc.psum_tensor("ps10", [8, 512], F32) as ps10,
        nc.psum_tensor("ps11", [8, 512], F32) as ps11,
        nc.semaphore("dsem") as dsem,
        nc.semaphore("g_done") as g_done,
        nc.semaphore("m_done") as m_done,
        nc.semaphore("c_done") as c_done,
        nc.semaphore("o_sem") as o_sem,
        nc.Block() as block,
    ):
        oA = [oA0, oA1]
        oB = [oB0, oB1]
        ps = [[ps00, ps01], [ps10, ps11]]
        stg = [stg0, stg1]
        total_t = repeat * NTILES
        total_b = repeat * NBLK

        @block.sync
        def _(sync):
            sync.dma_start(w_sb[:, :], w[:, :]).then_inc(dsem, 16)
            sync.dma_start(d0_sb[:, :], d0[:, :]).then_inc(dsem, 16)
            sync.dma_start(d1_sb[:, :], d1[:, :]).then_inc(dsem, 16)
            sync.dma_start(i_sb[:, :], idx[:, :]).then_inc(dsem, 16)
            for b in range(total_b):
                pb = b % 2
                sync.wait_ge(c_done, BLK * (b + 1))
                sync.dma_start(
                    bass.AP(out, STG_F * (b % NBLK), [[NPX, 8], [1, STG_F]]),
                    stg[pb][:, :],
                ).then_inc(o_sem, 16)
            sync.wait_ge(o_sem, 16 * total_b)

        @block.gpsimd
        def _(g):
            g.wait_ge(dsem, 64)
            for gi in range(total_t):
                i = gi % NTILES
                p = gi % 2
                if gi >= 2:
                    g.wait_ge(m_done, gi - 1)
                g.indirect_copy(
                    oA[p][:, :nv], d0_sb[:, :],
                    i_sb[:, 64 * i:64 * i + nv // 16], True)
                g.indirect_copy(
                    oB[p][:, :nv], d1_sb[:, :],
                    i_sb[:, IDX_F + 64 * i:IDX_F + 64 * i + nv // 16], True,
                ).then_inc(g_done, 1)

        @block.tensor
        def _(tensor):
            tensor.wait_ge(dsem, 64)
            for gi in range(total_t):
                p = gi % 2
                tensor.wait_ge(g_done, gi + 1)
                if gi >= 2:
                    tensor.wait_ge(c_done, gi - 1)
                h = nv // 2
                tensor.matmul(ps[p][0][:, :h], w_sb[:, :], oA[p][:, 0:h],
                              start=True, stop=False)
                tensor.matmul(ps[p][0][:, :h], w_sb[:, :], oB[p][:, 0:h],
                              start=False, stop=True)
                tensor.matmul(ps[p][1][:, :h], w_sb[:, :], oA[p][:, h:2 * h],
                              start=True, stop=False)
                tensor.matmul(ps[p][1][:, :h], w_sb[:, :], oB[p][:, h:2 * h],
                              start=False, stop=True).then_inc(m_done, 1)

        @block.scalar
        def _(scalar):
            for gi in range(total_t):
                p = gi % 2
                b = gi // BLK
                pb = b % 2
                scalar.wait_ge(m_done, gi + 1)
                if b >= 2 and gi % BLK == 0:
                    scalar.wait_ge(o_sem, 16 * (b - 1))
                f0 = (gi % BLK) * TILE
                h = nv // 2
                scalar.copy(stg[pb][:, f0:f0 + h], ps[p][0][:, :h])
                scalar.copy(stg[pb][:, f0 + 512:f0 + 512 + h],
                            ps[p][1][:, :h]).then_inc(c_done, 1)

    return nc


_NC_CACHE: dict = {}


def _get_nc(repeat: int = 1, nv: int = TILE) -> bass.Bass:
    if (repeat, nv) not in _NC_CACHE:
        _NC_CACHE[(repeat, nv)] = build_nc(repeat, nv)
    return _NC_CACHE[(repeat, nv)]


def make_in_maps(sensor_data: np.ndarray, t_u16: np.ndarray):
    """Per-core input dicts. t_u16: [128 sensors, NPX] uint16."""
    sd = np.asarray(sensor_data, dtype=np.float32)        # (4, 2, 128, 2048)
    traces = sd.transpose(2, 0, 1, 3).reshape(S, 8, T)    # (s, c=(b,c2), T)
    # W[16k+j, c] = (j == c), j < 8: selects channel c of each sensor group,
    # zeroing the replica rows (j >= 8).
    w = np.zeros((128, 8), np.float32)
    for k in range(8):
        for j in range(8):
            w[16 * k + j, j] = 1.0
    in_maps = []
    for n in range(NCORES):
        tn = t_u16[SPC * n:SPC * (n + 1)]                 # (16, NPX)
        idx = np.empty((128, 2 * IDX_F), np.uint16)
        d = np.empty((2, 128, T), np.float32)
        for g in range(2):
            for k in range(8):
                s_loc = 8 * g + k
                # tile i covers px [1024*i, 1024*(i+1)); stream wrapped over
                # the core's 16 partitions (partition index fastest).
                blk = (tn[s_loc].reshape(NTILES, 64, 16)
                       .transpose(2, 0, 1).reshape(16, IDX_F))
                idx[16 * k:16 * k + 16, g * IDX_F:(g + 1) * IDX_F] = blk
                for j in range(16):
                    d[g, 16 * k + j] = traces[SPC * n + s_loc, j % 8]
        in_maps.append({"w": w, "d0": d[0], "d1": d[1], "idx": idx})
    return in_maps


def kernel(sensor_data: np.ndarray, sensor_xy: np.ndarray) -> np.ndarray:
    t = _delay_indices(np.asarray(sensor_xy))
    t_u16 = t.astype(np.uint16)
    in_maps = make_in_maps(sensor_data, t_u16)
    nc = _get_nc(1)
    res = run_bass_kernel_spmd(nc, in_maps, list(range(NCORES)))
    acc = np.zeros((8, NPX), np.float64)
    for r in res.results:
        acc += r["out"]
    return acc.astype(np.float32).reshape(4, 2, NX, NY)



# revision 2
# speedup vs baseline: 1.2072x; 1.2072x over previous
"""DAS (delay-and-sum) beamforming on 8 Trainium2 NeuronCores.

Strategy: PE-matmul delay-block gather (replaces the GPSIMD indirect_copy
baseline, ~3x faster).

The delay index map t(u,w) = int(sqrt((u*DX)^2+(w*DY)^2)/VS/DT) is constant
geometry; a sensor at (x,y) merely windows it.  Pair := (sensor s, image
column ixi); U := x_s - ixi is shared across pairs, so for each U a constant
one-hot "delay block" B_U[128, 1024] (one-hot over tau mod 128 per rho
column, rho = 511 - w) turns the per-pixel gather into PE matmuls:

    psum[(q,c), rho] = sum_tau' data[tau', (s_q, chunk, c)] * B_U[tau', rho]

with the 2048-sample contraction split into 16 tau-chunks of 128; each rho
column belongs to exactly one chunk, so per (U, chunk-interval) a single
matmul (stationary = data slice [128, 16 sensors x 8 ch], moving = block
columns) writes a disjoint psum column range.  Pairs are grouped 16-at-a-time
by x-sorted sensor rank so the stationary is one contiguous slice.

Output: each psum row is a 1024-long image-row profile; indirect_dma_start
with compute_op=add scatters all 128 rows of a group into a padded per-core
image in HBM at per-row element offsets (shift by y_s).  DMA-add collisions
race, so same-ixi rows are kept >=2 output instructions apart (depth-2 DMA
pipeline + per-group conflict flags force deeper waits when needed).

Sharding: cores take contiguous U ranges balanced by group count (each core
builds its own program; compiles are cached per sensor_xy).  The host sums
the 8 padded partial images and crops to (4, 2, 512, 512).
"""

import numpy as np

import concourse.bass as bass
import concourse.mybir as mybir

F32 = mybir.dt.float32
I32 = mybir.dt.int32

NX = NY = 512
DX = DY = 1e-4
VS = 1550.0
DT = 2.5e-8

S = 128
T = 2048
NCHUNK = 16
NU = 1023            # U in [-511, 511]
NRHO = 1024          # rho = 511 - w, w in [-512, 511]
PADW = 1536          # padded image row width (cols y..y+1023, y <= 511)
NROW = 8 * 512       # (c, ixi) rows
DUMP_ROW = NROW      # scratch row for invalid pairs
NCORES = 8

# ---------------------------------------------------------------- constants

_CONST = {}


def _tables():
    if "T" in _CONST:
        return _CONST["T"], _CONST["CHUNK"], _CONST["BLOCKS"]
    U = np.arange(-511, 512, dtype=np.float32)[:, None]
    W = (511 - np.arange(NRHO, dtype=np.float32))[None, :]
    dx = (U * np.float32(DX)) ** 2
    dy = (W * np.float32(DY)) ** 2
    dis = np.sqrt(dx + dy, dtype=np.float32)
    tt = (dis / np.float32(VS) / np.float32(DT)).astype(np.int32)  # [1023,1024]
    chunk = (tt >> 7).astype(np.int8)
    blocks = np.zeros((NU, 128, NRHO), np.float32)
    ui = np.broadcast_to(np.arange(NU)[:, None], tt.shape)
    rho = np.broadcast_to(np.arange(NRHO)[None, :], tt.shape)
    blocks[ui, tt & 127, rho] = 1.0
    _CONST["T"] = tt
    _CONST["CHUNK"] = chunk
    _CONST["BLOCKS"] = blocks
    return tt, chunk, blocks


def _segs_for_u(chunkrow):
    """[(chunk, a, b)] covering rho [0,1024), split at the 512 boundary."""
    segs = []
    a = 0
    cur = int(chunkrow[0])
    for r in range(1, NRHO + 1):
        c = int(chunkrow[r]) if r < NRHO else -1
        if c != cur or r == NRHO:
            lo, hi = a, r
            if lo < 512 < hi:
                segs.append((cur, lo, 512))
                segs.append((cur, 512, hi))
            else:
                segs.append((cur, lo, hi))
            a, cur = r, c
    return segs


# ---------------------------------------------------------------- planning


def _perm_ranks(x):
    """x-sorted sensor order; spread equal-x sensors far apart in rank so
    same-ixi pairs never share an output instruction."""
    order = list(np.argsort(x, kind="stable"))
    xs = [int(x[o]) for o in order]
    for i in range(S):
        j = 1
        while i + j < S and xs[i + j] == xs[i]:
            tgt = i + j + 32 * j
            if tgt >= S:
                tgt = max(0, i - 32 * j)
            order[i + j], order[tgt] = order[tgt], order[i + j]
            xs[i + j], xs[tgt] = xs[tgt], xs[i + j]
            j += 1
    return np.array(order)


def make_plan(sensor_xy):
    tt, chunktab, _ = _tables()
    xy = np.asarray(sensor_xy)
    order = _perm_ranks(xy[:, 0])
    xs = xy[order, 0].astype(np.int64)
    ys = xy[order, 1].astype(np.int64)

    u_entries = []   # (U, [(k, valid16) ...])
    for U in range(-511, 512):
        ixi = xs - U
        valid = (ixi >= 0) & (ixi < 512)
        if not valid.any():
            continue
        ks = []
        for k in range(8):
            v16 = valid[16 * k:16 * k + 16]
            if v16.any():
                ks.append((k, v16.copy()))
        u_entries.append((U, ks))

    # split U entries into 8 contiguous runs with ~equal group counts
    costs = np.array([len(ks) + 0.35 for _, ks in u_entries])
    cum = np.cumsum(costs)
    total = cum[-1]
    bounds = [0]
    for c in range(1, NCORES):
        bounds.append(int(np.searchsorted(cum, total * c / NCORES)))
    bounds.append(len(u_entries))

    cores = []
    for ci in range(NCORES):
        ents = u_entries[bounds[ci]:bounds[ci + 1]]
        u_list, seg_list, groups, gend = [], [], [], []
        for U, ks in ents:
            ui = len(u_list)
            u_list.append(U)
            seg_list.append(_segs_for_u(chunktab[U + 511]))
            for k, v16 in ks:
                off = np.full(128, DUMP_ROW * PADW, np.int64)
                ixis = set()
                for q in range(16):
                    r = 16 * k + q
                    if v16[q]:
                        ixi = int(xs[r] - U)
                        ixis.add(ixi)
                        for c in range(8):
                            off[q * 8 + c] = (c * 512 + ixi) * PADW + ys[r]
                groups.append((ui, k, off.astype(np.int32), ixis))
            gend.append(len(groups))
        # adjacent output instructions must touch disjoint image rows
        deep = [False] * len(groups)
        for g in range(1, len(groups)):
            if groups[g][3] & groups[g - 1][3]:
                deep[g] = True
        cores.append(dict(u_list=u_list, seg_list=seg_list, groups=groups,
                          gend=gend, deep=deep))
    return dict(order=order, xs=xs, ys=ys, cores=cores)


# ---------------------------------------------------------------- bass build


def build_core_nc(plan_core, repeat=1):
    u_list = plan_core["u_list"]
    seg_list = plan_core["seg_list"]
    groups = plan_core["groups"]
    gend = plan_core["gend"]
    deep = plan_core["deep"]
    nU = len(u_list)
    G = len(groups)
    GT = repeat * G

    nc = bass.Bass()
    data = nc.declare_dram_parameter("data", [128, NCHUNK * 128 * 8], F32,
                                     isOutput=False)
    blocks = nc.declare_dram_parameter("blocks", [nU * 128, NRHO], F32,
                                       isOutput=False)
    offs = nc.declare_dram_parameter("offs", [128, G], I32, isOutput=False)
    img = nc.declare_dram_parameter("img", [(NROW + 1) * PADW, 1], F32,
                                    isOutput=True)

    with (
        nc.sbuf_tensor("data_sb", [128, NCHUNK * 128 * 8], F32) as data_sb,
        nc.sbuf_tensor("blk0", [128, NRHO], F32) as blk0,
        nc.sbuf_tensor("blk1", [128, NRHO], F32) as blk1,
        nc.sbuf_tensor("offs_sb", [128, G], I32) as offs_sb,
        nc.sbuf_tensor("stg0", [128, NRHO], F32) as stg0,
        nc.sbuf_tensor("stg1", [128, NRHO], F32) as stg1,
        nc.sbuf_tensor("stg2", [128, NRHO], F32) as stg2,
        nc.sbuf_tensor("stg3", [128, NRHO], F32) as stg3,
        nc.psum_tensor("ps00", [128, 512], F32) as ps00,
        nc.psum_tensor("ps01", [128, 512], F32) as ps01,
        nc.psum_tensor("ps10", [128, 512], F32) as ps10,
        nc.psum_tensor("ps11", [128, 512], F32) as ps11,
        nc.semaphore("d_sem") as d_sem,
        nc.semaphore("b_sem") as b_sem,
        nc.semaphore("m_done") as m_done,
        nc.semaphore("a_done") as a_done,
        nc.semaphore("v_done") as v_done,
        nc.semaphore("o_sem") as o_sem,
        nc.Block() as block,
    ):
        blk = [blk0, blk1]
        stg = [stg0, stg1, stg2, stg3]
        ps = [[ps00, ps01], [ps10, ps11]]

        @block.sync
        def _(sync):
            sync.dma_start(data_sb[:, :], data[:, :]).then_inc(d_sem, 16)
            sync.dma_start(offs_sb[:, :], offs[:, :]).then_inc(d_sem, 16)
            for rep in range(repeat):
                for ui in range(nU):
                    ug = rep * nU + ui
                    if ug >= 2:
                        pu = ug - 2
                        pg = (pu // nU) * G + gend[pu % nU]
                        sync.wait_ge(m_done, pg)
                    sync.dma_start(
                        blk[ug % 2][:, :],
                        blocks[ui * 128:(ui + 1) * 128, :],
                    ).then_inc(b_sem, 16)
            sync.wait_ge(o_sem, 16 * GT)

        @block.tensor
        def _(tensor):
            tensor.wait_ge(d_sem, 32)
            for rep in range(repeat):
                for ui in range(nU):
                    ug = rep * nU + ui
                    tensor.wait_ge(b_sem, 16 * (ug + 1))
                    segs = seg_list[ui]
                    g0 = rep * G + (gend[ui - 1] if ui > 0 else 0)
                    g1 = rep * G + gend[ui]
                    for g in range(g0, g1):
                        _, k, _, _ = groups[g % G]
                        if g >= 2:
                            tensor.wait_ge(a_done, g - 1)
                            tensor.wait_ge(v_done, g - 1)
                        mm = None
                        for (ch, a, b) in segs:
                            half = 1 if a >= 512 else 0
                            mm = tensor.matmul(
                                ps[g % 2][half][:,
                                                a - 512 * half:b - 512 * half],
                                data_sb[:, ch * 1024 + k * 128:
                                        ch * 1024 + k * 128 + 128],
                                blk[ug % 2][:, a:b],
                                start=True, stop=True,
                            )
                        mm.then_inc(m_done, 1)

        @block.scalar
        def _(scalar):
            for g in range(GT):
                scalar.wait_ge(m_done, g + 1)
                if g >= 4:
                    scalar.wait_ge(o_sem, 16 * (g - 3))
                scalar.copy(stg[g % 4][:, 0:512],
                            ps[g % 2][0][:, :]).then_inc(a_done, 1)

        @block.vector
        def _(vector):
            for g in range(GT):
                vector.wait_ge(m_done, g + 1)
                if g >= 4:
                    vector.wait_ge(o_sem, 16 * (g - 3))
                vector.tensor_copy(out=stg[g % 4][:, 512:1024],
                                   in_=ps[g % 2][1][:, :]).then_inc(v_done, 1)

        @block.gpsimd
        def _(gp):
            for g in range(GT):
                gp.wait_ge(a_done, g + 1)
                gp.wait_ge(v_done, g + 1)
                lim = g if deep[g % G] else g - 1
                if lim >= 1:
                    gp.wait_ge(o_sem, 16 * lim)
                gp.indirect_dma_start(
                    out=img[:, :],
                    out_offset=bass.IndirectOffsetOnAxis(
                        ap=offs_sb[:, (g % G):(g % G) + 1], axis=0),
                    in_=stg[g % 4][:, :],
                    in_offset=None,
                    compute_op=mybir.AluOpType.add,
                ).then_inc(o_sem, 16)

    return nc


# ---------------------------------------------------------------- runner


class _CoreRunner:
    """Execute one prebuilt single-core Bass module via PJRT with a cached
    jitted callable (compiles once, reruns cheaply)."""

    def __init__(self, nc, device):
        import jax
        from concourse.bass2jax import (
            _bass_exec_p, install_neuronx_cc_hook, partition_id_tensor,
        )

        install_neuronx_cc_hook()
        self.device = device
        partition_name = (
            nc.partition_id_tensor.name if nc.partition_id_tensor else None
        )
        in_names, out_names, out_avals, zero_shapes = [], [], [], []
        for alloc in nc.m.functions[0].allocations:
            if not isinstance(alloc, mybir.MemoryLocationSet):
                continue
            name = alloc.memorylocations[0].name
            if alloc.kind == "ExternalInput":
                if name != partition_name:
                    in_names.append(name)
            elif alloc.kind == "ExternalOutput":
                shape = tuple(alloc.tensor_shape)
                dtype = mybir.dt.np(alloc.dtype)
                out_names.append(name)
                out_avals.append(jax.core.ShapedArray(shape, dtype))
                zero_shapes.append((shape, dtype))
        self.in_names = in_names
        self.out_names = out_names
        self.zero_shapes = zero_shapes
        n_params = len(in_names)
        n_outs = len(out_avals)
        all_in_names = in_names + out_names
        if partition_name is not None:
            all_in_names.append(partition_name)
        donate = tuple(range(n_params, n_params + n_outs))

        def _body(*args):
            operands = list(args)
            if partition_name is not None:
                operands.append(partition_id_tensor())
            outs = _bass_exec_p.bind(
                *operands,
                out_avals=tuple(out_avals),
                in_names=tuple(all_in_names),
                out_names=tuple(out_names),
                lowering_input_output_aliases=(),
                sim_require_finite=True,
                sim_require_nnan=True,
                nc=nc,
            )
            return tuple(outs)

        self._fn = jax.jit(_body, donate_argnums=donate, keep_unused=True)

    def launch(self, in_map):
        import jax

        cin = [jax.device_put(np.asarray(in_map[n]), self.device)
               for n in self.in_names]
        zouts = [jax.device_put(np.zeros(s, d), self.device)
                 for (s, d) in self.zero_shapes]
        return self._fn(*cin, *zouts)


# ---------------------------------------------------------------- host side


def make_in_maps(sensor_data, plan):
    _, _, blocks_all = _tables()
    sd = np.asarray(sensor_data, np.float32)          # (4,2,128,2048)
    tr = sd.reshape(8, S, T)[:, plan["order"], :]      # (c, rank, T)
    d = tr.reshape(8, S, NCHUNK, 128)                  # c, rank, chunk, tau'
    d = d.transpose(3, 2, 1, 0).reshape(128, NCHUNK * S * 8).copy()
    in_maps = []
    for ci in range(NCORES):
        pc = plan["cores"][ci]
        uidx = np.array(pc["u_list"]) + 511
        blk = blocks_all[uidx].reshape(len(uidx) * 128, NRHO)
        offs = np.stack([g[2] for g in pc["groups"]], axis=1)  # [128, G]
        in_maps.append({"data": d, "blocks": blk, "offs": offs})
    return in_maps


_RUN_CACHE = {}


def _get_runners(plan_key, plan, repeat=1):
    import jax

    key = (plan_key, repeat)
    if key not in _RUN_CACHE:
        devs = jax.devices()
        runners = []
        for ci in range(NCORES):
            nc = build_core_nc(plan["cores"][ci], repeat)
            runners.append(_CoreRunner(nc, devs[ci]))
        _RUN_CACHE[key] = runners
    return _RUN_CACHE[key]


def kernel(sensor_data, sensor_xy):
    plan_key = np.asarray(sensor_xy).tobytes()
    plan = make_plan(sensor_xy)
    in_maps = make_in_maps(sensor_data, plan)
    runners = _get_runners(plan_key, plan)

    outs = [runners[ci].launch(in_maps[ci]) for ci in range(NCORES)]
    acc = np.zeros((8, 512, 512), np.float64)
    for ci in range(NCORES):
        img = np.asarray(outs[ci][0]).reshape(NROW + 1, PADW)
        acc += img[:NROW, 511:1023].reshape(8, 512, 512)
    return acc.astype(np.float32).reshape(4, 2, 512, 512)
